# revision 1
# baseline (speedup 1.0000x reference)
"""Trainium2 Bass kernel for CTNNBackflowNet forward (gnn_message_passing).

B=16, N=128, D=3, H=128.  Data-parallel: 2 samples/core x 8 NeuronCores, no
collectives (forward only).  Raw Bass Block style with explicit semaphores
(standalone wait_ge instructions -- this walrus build rejects instructions
carrying several attached sync waits, which rules out the Tile scheduler).

Layout: feature/H axis on SBUF partitions, electron j on the free axis.
Algebraic restructurings vs the reference:
  * rev_w hoisted past the masked j-sum:  m_v = (sum_{j!=i} h_e_new) @ rev_w.
  * edge_update concat([h_e, v_i, v_j]) @ eu1_w  splits into U1^T h_e
    + (U2^T vie)_i + (U3^T vie)_j, the latter two computed once per sample
    and injected into PSUM via identity-weight matmuls with broadcast APs.
  * r2 = |x_i|^2 + |x_j|^2 - 2 x_i.x_j via a tiny Gram matmul per 16-electron
    block (fp32 difference path to avoid cancellation).
  * The 5-feature edge layer-1 is one K=128 matmul per electron against a
    block tile packing [x_j-x_i | r1 | r2] at partition bases {0,64,96},
    with host-built per-slot selector weights.
N^2-sized matmuls run in bf16 (fp32 PSUM accumulate); everything touching
raw coordinates or the final output stays fp32.
"""

import numpy as np
import ml_dtypes

B, N, D = 16, 128, 3
H = 128
EPS = 1e-12
NCORES = 8
BPC = B // NCORES
IBLK = 16
NBLK = N // IBLK
GRP = 4
NGRP = IBLK // GRP

_CACHE = {}
SIM_COMPAT = False  # decompose Silu for CoreSim validation


def _build_nc():
    import concourse.bass as bass
    import concourse.mybir as mybir
    from contextlib import ExitStack

    f32 = mybir.dt.float32
    bf16 = mybir.dt.bfloat16
    AF = mybir.ActivationFunctionType
    ALU = mybir.AluOpType

    nc = bass.Bass()
    P = {}

    def par(name, shape, dt=f32):
        P[name] = nc.declare_dram_parameter(name, list(shape), dt, isOutput=False)
        return P[name]

    par("xt4", (BPC, 4, N))
    par("xtrep", (BPC, 48, N))
    par("xiall", (BPC, 48, NBLK))
    par("ne_w", (4, H)); par("ne_b", (H, 1))
    par("ee1_b", (H, 1)); par("ee2_b", (H, 1))
    par("rve_w", (H, H)); par("rev_w", (H, H))
    par("eu1_w_f", (3, H, H))
    par("eu1_b", (H, 1)); par("eu2_b", (H, 1))
    par("nu1_w", (2, H, H)); par("nu1_b", (H, 1))
    par("nu2_w", (H, H)); par("nu2_b", (H, 1))
    par("nu3_w", (H, H)); par("nu3_b", (H, 1))
    par("dx_w", (H, D)); par("dx_b", (D, 1))
    par("bf_raw", (1, 1))
    par("sel", (IBLK, 128, H), bf16)
    par("ee2_w", (H, H), bf16)
    par("U1b", (H, H), bf16)
    par("eu2_w", (H, H), bf16)
    par("identb", (128, 128), bf16)
    out_ext = nc.declare_dram_parameter("out", [BPC, D, N], f32, isOutput=True)

    ctx = ExitStack()

    def sb(name, shape, dt=f32):
        return ctx.enter_context(nc.sbuf_tensor('s_' + name, list(shape), dt))

    def ps(name, shape):
        return ctx.enter_context(nc.psum_tensor('ps_' + name, list(shape), f32))

    with ctx:
        xt4 = [sb(f"xt4_{s}", (4, N)) for s in range(BPC)]
        xtrep = [sb(f"xtrep_{s}", (48, N)) for s in range(BPC)]
        xi_all = [sb(f"xi_{s}", (48, NBLK)) for s in range(BPC)]
        ne_w = sb("ne_w", (4, H)); ne_b = sb("ne_b", (H, 1))
        ee1_b = sb("ee1_b", (H, 1)); ee2_b = sb("ee2_b", (H, 1))
        rve_w = sb("rve_w", (H, H)); rev_w = sb("rev_w", (H, H))
        U1f = sb("U1f", (H, H)); U2f = sb("U2f", (H, H)); U3f = sb("U3f", (H, H))
        eu1_b = sb("eu1_b", (H, 1)); eu2_b = sb("eu2_b", (H, 1))
        nu1a = sb("nu1a", (H, H)); nu1b = sb("nu1b", (H, H))
        nu1_bc = sb("nu1_bc", (H, 1))
        nu2_w = sb("nu2_w", (H, H)); nu2_bc = sb("nu2_bc", (H, 1))
        nu3_w = sb("nu3_w", (H, H)); nu3_bc = sb("nu3_bc", (H, 1))
        dx_w = sb("dx_w", (H, D)); dx_bc = sb("dx_bc", (D, 1))
        bf3 = sb("bf3", (3, 1))
        sel = [sb(f"sel_{a}", (128, H), bf16) for a in range(IBLK)]
        ee2_wb = sb("ee2_wb", (H, H), bf16)
        U1b = sb("U1b", (H, H), bf16)
        eu2_wb = sb("eu2_wb", (H, H), bf16)
        identb = sb("identb", (128, 128), bf16)

        ones31 = sb("ones31", (3, 1))
        ones_row = sb("ones_row", (1, N))
        eps16 = sb("eps16", (IBLK, 1))
        zcol = sb("zcol", (H, 1))
        b127 = sb("b127", (H, 1))
        sp3 = sb("sp3", (3, 1))

        n2xt = [sb(f"n2xt_{s}", (3, N)) for s in range(BPC)]
        rsqT = [sb(f"rsqT_{s}", (3, N)) for s in range(BPC)]
        sq_row = [sb(f"sq_row_{s}", (1, N)) for s in range(BPC)]
        h_vT = [sb(f"h_vT_{s}", (H, N)) for s in range(BPC)]
        vieT = [sb(f"vieT_{s}", (H, N)) for s in range(BPC)]
        cc2 = [sb(f"cc2_{s}", (H, 1)) for s in range(BPC)]
        t3b = [sb(f"t3b_{s}", (H, N), bf16) for s in range(BPC)]
        C2b = [sb(f"C2b_{s}", (H, N), bf16) for s in range(BPC)]
        S_T = [sb(f"S_T_{s}", (H, N)) for s in range(BPC)]
        m_v = [sb(f"m_v_{s}", (H, N)) for s in range(BPC)]
        a1 = [sb(f"a1_{s}", (H, N)) for s in range(BPC)]
        a2 = [sb(f"a2_{s}", (H, N)) for s in range(BPC)]
        dh = [sb(f"dh_{s}", (H, N)) for s in range(BPC)]
        hnew = [sb(f"hnew_{s}", (H, N)) for s in range(BPC)]
        dxT = [sb(f"dxT_{s}", (D, N)) for s in range(BPC)]
        mu = [sb(f"mu_{s}", (D, 1)) for s in range(BPC)]
        mus = [sb(f"mus_{s}", (D, 1)) for s in range(BPC)]
        dxo = [sb(f"dxo_{s}", (D, N)) for s in range(BPC)]

        E_slots = [sb(f"eslot{e}", (128, N), bf16) for e in range(3)]
        he1_t = [sb(f"he1_{p}", (H, GRP * N), bf16) for p in range(2)]
        he_t = [sb(f"he_{p}", (H, GRP * N), bf16) for p in range(2)]
        heu1_t = [sb(f"heu1_{p}", (H, GRP * N), bf16) for p in range(2)]
        sums_t = [sb(f"sums_{p}", (H, GRP)) for p in range(2)]
        silu_s = ([sb(f"silu_{p}", (H, GRP * N)) for p in range(2)]
                  if SIM_COMPAT else None)

        ppre1 = [ps(f"ppre1_{p}", (128, 512)) for p in range(2)]
        pp2 = ps("pp2", (128, 512))
        ppeu = ps("ppeu", (128, 512))
        ppeu2 = [ps(f"ppeu2_{p}", (128, 512)) for p in range(2)]
        psm = [ps(f"psm_{p}", (128, 512)) for p in range(2)]

        OPS = []

        def op(engine, emit, deps=(), key=None, sem=None):
            OPS.append((engine, emit, list(deps), key, sem))

        def dma(dst, src, deps=(), key=None, cls="dma_w"):
            op("sync", lambda e, d=dst, s=src: e.dma_start(out=d, in_=s), deps,
               key, sem=cls)

        dma(ne_w[:], P["ne_w"][:]); dma(ne_b[:], P["ne_b"][:])
        dma(ee1_b[:], P["ee1_b"][:]); dma(ee2_b[:], P["ee2_b"][:], key="d_ee2b")
        dma(rve_w[:], P["rve_w"][:]); dma(rev_w[:], P["rev_w"][:])
        dma(U1f[:], P["eu1_w_f"][0]); dma(U2f[:], P["eu1_w_f"][1])
        dma(U3f[:], P["eu1_w_f"][2])
        dma(eu1_b[:], P["eu1_b"][:], key="d_eu1b")
        dma(eu2_b[:], P["eu2_b"][:], key="d_eu2b")
        dma(nu1a[:], P["nu1_w"][0]); dma(nu1b[:], P["nu1_w"][1])
        dma(nu1_bc[:], P["nu1_b"][:])
        dma(nu2_w[:], P["nu2_w"][:]); dma(nu2_bc[:], P["nu2_b"][:])
        dma(nu3_w[:], P["nu3_w"][:]); dma(nu3_bc[:], P["nu3_b"][:])
        dma(dx_w[:], P["dx_w"][:]); dma(dx_bc[:], P["dx_b"][:])
        bfap = P["bf_raw"][:]
        dma(bf3[:], bass.AP(tensor=bfap.tensor, offset=bfap.offset,
                            ap=[[0, 3], [1, 1]]), key="d_bf")
        for a in range(IBLK):
            dma(sel[a][:], P["sel"][a])
        dma(ee2_wb[:], P["ee2_w"][:]); dma(U1b[:], P["U1b"][:])
        dma(eu2_wb[:], P["eu2_w"][:]); dma(identb[:], P["identb"][:])
        for s in range(BPC):
            dma(xt4[s][:], P["xt4"][s], key=f"d_xt4_{s}", cls="dma_x")
            dma(xtrep[s][:], P["xtrep"][s], key=f"d_xtrep_{s}", cls="dma_x")
            dma(xi_all[s][:], P["xiall"][s], key=f"d_xi_{s}", cls="dma_x")

        op("dve", lambda e: e.memset(ones31[:], 1.0))
        op("dve", lambda e: e.memset(ones_row[:], 1.0))
        op("dve", lambda e: e.memset(eps16[:], EPS))
        op("dve", lambda e: e.memset(zcol[:], 0.0), key="k_z")
        for ei in range(3):
            op("dve", lambda e, ei=ei: e.memset(E_slots[ei][:], 0.0),
               key=f"k_ez_{ei}")
        op("act", lambda e: e.mul(out=b127[:], in_=eu2_b[:], mul=float(N - 1)),
           deps=[("dma_w", "TOTAL")], key="k_b127")
        op("act", lambda e: e.activation(out=sp3[:], in_=bf3[:], func=AF.Exp),
           deps=[("dma_w", "TOTAL")], key="k_spe")
        op("dve", lambda e: e.tensor_scalar_add(out=sp3[:], in0=sp3[:], scalar1=1.0),
           deps=[("act", "k_spe")], key="k_sp1")
        op("act", lambda e: e.activation(out=sp3[:], in_=sp3[:], func=AF.Ln),
           deps=[("dve", "k_sp1")], key="k_sp3")

        SILU_ENG = "dve" if SIM_COMPAT else "act"

        def silu(out_ap, in_ap, bias_ap, shape, parity, key, deps):
            if not SIM_COMPAT:
                op("act", lambda e: e.activation(out=out_ap, in_=in_ap,
                                                 func=AF.Silu, bias=bias_ap),
                   deps=deps, key=key)
            else:
                sc = silu_s[parity][0:shape[0], 0:shape[1]]
                op("act", lambda e: e.activation(out=sc, in_=in_ap,
                                                 func=AF.Sigmoid, bias=bias_ap),
                   deps=deps)
                op("act", lambda e: e.activation(out=out_ap, in_=in_ap,
                                                 func=AF.Identity, bias=bias_ap),
                   key=key + "_i")
                op("dve", lambda e: e.tensor_mul(out=out_ap, in0=out_ap, in1=sc),
                   deps=list(deps) + [("act", key + "_i")], key=key)

        for s in range(BPC):
            prevw = []
            if s > 0:
                prevw = [("act", f"k_dxT_{s - 1}")]
            op("act", lambda e, s=s: e.activation(out=n2xt[s][:], in_=xt4[s][0:3, :],
                                                  func=AF.Copy, scale=-2.0),
               deps=[("dma_x", "TOTAL")], key=f"k_n2xt_{s}")
            op("dve", lambda e, s=s: e.tensor_mul(out=rsqT[s][:], in0=xt4[s][0:3, :],
                                                  in1=xt4[s][0:3, :]),
               deps=[("dma_x", "TOTAL"), ("dve", "k_z")], key=f"k_rsq_{s}")
            op("pe", lambda e, s=s: e.matmul(psm[0][0:1, 0:N], ones31[:], rsqT[s][:],
                                             start=True, stop=True),
               deps=[("dve", f"k_rsq_{s}")] + prevw, key=f"p_sq_{s}")
            op("dve", lambda e, s=s: e.tensor_copy(out=sq_row[s][:],
                                                   in_=psm[0][0:1, 0:N]),
               deps=[("pe", f"p_sq_{s}")], key=f"k_sqrow_{s}")
            prevw2 = [("act", f"k_dh_{s - 1}")] if s > 0 else []
            op("pe", lambda e, s=s: e.matmul(psm[1][0:H, 0:N], ne_w[:], xt4[s][:],
                                             start=True, stop=True),
               deps=[("dma_w", "TOTAL")] + prevw2, key=f"p_hv_{s}")
            op("act", lambda e, s=s: e.activation(out=h_vT[s][:], in_=psm[1][0:H, 0:N],
                                                  func=AF.Identity, bias=ne_b[:]),
               deps=[("pe", f"p_hv_{s}")], key=f"k_hv_{s}")
            op("pe", lambda e, s=s: e.matmul(psm[0][0:H, 0:N], rve_w[:], h_vT[s][:],
                                             start=True, stop=True),
               deps=[("act", f"k_hv_{s}"), ("dve", f"k_sqrow_{s}")], key=f"p_vie_{s}")
            op("dve", lambda e, s=s: e.tensor_copy(out=vieT[s][:],
                                                   in_=psm[0][0:H, 0:N]),
               deps=[("pe", f"p_vie_{s}")], key=f"k_vie_{s}")
            op("pe", lambda e, s=s: e.matmul(psm[1][0:H, 0:1], U1f[:], ee2_b[:],
                                             start=True, stop=True),
               deps=[("act", f"k_hv_{s}")], key=f"p_cc_{s}")
            op("dve", lambda e, s=s: e.tensor_tensor(out=cc2[s][:],
                                                     in0=psm[1][0:H, 0:1],
                                                     in1=eu1_b[:], op=ALU.add),
               deps=[("pe", f"p_cc_{s}")], key=f"k_cc_{s}")
            op("pe", lambda e, s=s: e.matmul(psm[1][0:H, 0:N], U3f[:], vieT[s][:],
                                             start=True, stop=True),
               deps=[("dve", f"k_cc_{s}"), ("dve", f"k_vie_{s}")], key=f"p_t3_{s}")
            op("act", lambda e, s=s: e.activation(out=t3b[s][:], in_=psm[1][0:H, 0:N],
                                                  func=AF.Identity, bias=cc2[s][:]),
               deps=[("pe", f"p_t3_{s}")], key=f"k_t3_{s}")
            op("pe", lambda e, s=s: e.matmul(psm[0][0:H, 0:N], U2f[:], vieT[s][:],
                                             start=True, stop=True),
               deps=[("act", f"k_t3_{s}"), ("dve", f"k_vie_{s}")], key=f"p_c2_{s}")
            op("dve", lambda e, s=s: e.tensor_copy(out=C2b[s][:], in_=psm[0][0:H, 0:N]),
               deps=[("pe", f"p_c2_{s}")], key=f"k_c2_{s}")

            for b in range(NBLK):
                i0 = b * IBLK
                gb = s * NBLK + b
                E = E_slots[gb % 3]
                edeps = [("dma_x", "TOTAL")]
                if gb >= 3:
                    edeps.append(("pe", f"p_pre1_{gb - 3}_{NGRP - 1}"))
                else:
                    edeps.append(("dve", f"k_ez_{gb % 3}"))
                op("dve", lambda e, s=s, b=b, E=E: e.tensor_tensor(
                    out=E[0:48, :], in0=xtrep[s][:],
                    in1=xi_all[s][:, b:b + 1].to_broadcast((48, N)),
                    op=ALU.subtract), deps=edeps, key=f"k_er_{gb}")
                pg = psm[gb % 2]
                pgdeps = [("act", f"k_n2xt_{s}"), ("dve", f"k_sqrow_{s}")]
                if gb >= 2:
                    pgdeps += [("dve", f"k_r2_{gb - 2}"), ("act", f"k_r1_{gb - 2}")]
                if b < 2:
                    # bank last touched by this sample's prep-phase readers
                    pgdeps += [("dve", f"k_c2_{s}"), ("act", f"k_t3_{s}")]
                op("pe", lambda e, s=s, i0=i0, pg=pg: e.matmul(
                    pg[0:IBLK, 0:N], n2xt[s][:, i0:i0 + IBLK], xt4[s][0:3, :],
                    start=True, stop=False), deps=pgdeps)
                op("pe", lambda e, s=s, pg=pg: e.matmul(
                    pg[0:IBLK, 0:N], ones_row[:, 0:IBLK], sq_row[s][:],
                    start=False, stop=False))
                op("pe", lambda e, s=s, i0=i0, pg=pg: e.matmul(
                    pg[0:IBLK, 0:N], sq_row[s][:, i0:i0 + IBLK], ones_row[:],
                    start=False, stop=True), key=f"p_g_{gb}")
                op("dve", lambda e, E=E, pg=pg: e.tensor_scalar(
                    out=E[96:96 + IBLK, :], in0=pg[0:IBLK, 0:N], scalar1=0.0,
                    scalar2=None, op0=ALU.max),
                   deps=[("pe", f"p_g_{gb}")], key=f"k_r2_{gb}")
                op("act", lambda e, E=E: e.activation(
                    out=E[64:64 + IBLK, :], in_=E[96:96 + IBLK, :],
                    func=AF.Sqrt, bias=eps16[:]),
                   deps=[("dve", f"k_r2_{gb}")], key=f"k_r1_{gb}")

                for g in range(NGRP):
                    gg = gb * NGRP + g
                    pp = gg % 2
                    p1 = ppre1[pp]
                    predeps = [("dve", f"k_er_{gb}"), ("act", f"k_r1_{gb}"),
                               ("dma_w", "TOTAL")]
                    if gg >= 2:
                        predeps.append((SILU_ENG, f"k_he1_{gg - 2}"))
                    for a4 in range(GRP):
                        a = g * GRP + a4
                        op("pe", lambda e, a=a, a4=a4, E=E, p1=p1: e.matmul(
                            p1[:, a4 * N:(a4 + 1) * N], sel[a][:], E[:],
                            start=True, stop=True),
                           deps=(predeps if a4 == 0 else ()),
                           key=(f"p_pre1_{gb}_{g}" if a4 == GRP - 1 else None))
                    silu(he1_t[pp][:], p1[:], ee1_b[:], (H, GRP * N), pp,
                         f"k_he1_{gg}", [("pe", f"p_pre1_{gb}_{g}")])
                    p2deps = [(SILU_ENG, f"k_he1_{gg}")]
                    if gg >= 1:
                        p2deps.append(("dve" if (gg - 1) % 2 == 0 else "act",
                                       f"k_he_{gg - 1}"))
                    op("pe", lambda e, pp=pp: e.matmul(
                        pp2[:], ee2_wb[:], he1_t[pp][:], start=True, stop=True),
                       deps=p2deps, key=f"p_p2_{gg}")
                    ceng = "dve" if gg % 2 == 0 else "act"
                    if ceng == "dve":
                        op("dve", lambda e, pp=pp: e.tensor_copy(
                            out=he_t[pp][:], in_=pp2[:]),
                           deps=[("pe", f"p_p2_{gg}")], key=f"k_he_{gg}")
                    else:
                        op("act", lambda e, pp=pp: e.activation(
                            out=he_t[pp][:], in_=pp2[:], func=AF.Copy),
                           deps=[("pe", f"p_p2_{gg}")], key=f"k_he_{gg}")
                    c0 = i0 + g * GRP
                    eudeps = [(ceng, f"k_he_{gg}"), ("act", f"k_t3_{s}"),
                              ("dve", f"k_c2_{s}")]
                    if gg >= 1:
                        eudeps.append((SILU_ENG, f"k_heu1_{gg - 1}"))
                    op("pe", lambda e, pp=pp: e.matmul(
                        ppeu[:], U1b[:], he_t[pp][:], start=True, stop=False),
                       deps=eudeps)
                    op("pe", lambda e, s=s: e.matmul(
                        ppeu[:], identb[:],
                        t3b[s][:, None, :].to_broadcast((H, GRP, N)),
                        start=False, stop=False))
                    op("pe", lambda e, s=s, c0=c0: e.matmul(
                        ppeu[:], identb[:],
                        C2b[s][:, c0:c0 + GRP, None].to_broadcast((H, GRP, N)),
                        start=False, stop=True), key=f"p_eu_{gg}")
                    silu(heu1_t[pp][:], ppeu[:], zcol[:], (H, GRP * N), pp,
                         f"k_heu1_{gg}", [("pe", f"p_eu_{gg}")])
                    eu2deps = [(SILU_ENG, f"k_heu1_{gg}")]
                    if gg >= 2:
                        eu2deps.append(("dve", f"k_st_{gg - 2}"))
                    op("pe", lambda e, pp=pp: e.matmul(
                        ppeu2[pp][:], eu2_wb[:], heu1_t[pp][:],
                        start=True, stop=True),
                       deps=eu2deps, key=f"p_eu2_{gg}")
                    sumdeps = [("pe", f"p_eu2_{gg}"), ("act", "k_b127")]
                    if gg >= 2:
                        sumdeps.append(("dve", f"k_sub_{gg - 2}"))
                    op("dve", lambda e, pp=pp: e.reduce_sum(
                        out=sums_t[pp][:],
                        in_=ppeu2[pp][:].rearrange("p (g j) -> p g j", j=N),
                        axis=mybir.AxisListType.X),
                       deps=sumdeps, key=f"k_sum_{gg}")
                    op("dve", lambda e, s=s, pp=pp, c0=c0: e.tensor_tensor(
                        out=S_T[s][:, c0:c0 + GRP], in0=sums_t[pp][:],
                        in1=ppeu2[pp][:, c0:c0 + 3 * (N + 1) + 1:N + 1],
                        op=ALU.subtract),
                       deps=[("dve", f"k_sum_{gg}")], key=f"k_sub_{gg}")
                    op("dve", lambda e, s=s, c0=c0: e.tensor_tensor(
                        out=S_T[s][:, c0:c0 + GRP], in0=S_T[s][:, c0:c0 + GRP],
                        in1=b127[:].to_broadcast((H, GRP)), op=ALU.add),
                       deps=[("dve", f"k_sub_{gg}")], key=f"k_st_{gg}")

            last_g = (s * NBLK + NBLK - 1) * NGRP + NGRP - 1
            op("pe", lambda e, s=s: e.matmul(psm[0][0:H, 0:N], rev_w[:], S_T[s][:],
                                             start=True, stop=True),
               deps=[("dve", f"k_st_{last_g}")], key=f"p_mv_{s}")
            op("dve", lambda e, s=s: e.tensor_copy(out=m_v[s][:],
                                                   in_=psm[0][0:H, 0:N]),
               deps=[("pe", f"p_mv_{s}")], key=f"k_mv_{s}")
            op("pe", lambda e, s=s: e.matmul(psm[1][0:H, 0:N], nu1a[:], h_vT[s][:],
                                             start=True, stop=False),
               deps=[("dve", f"k_mv_{s}")])
            op("pe", lambda e, s=s: e.matmul(psm[1][0:H, 0:N], nu1b[:], m_v[s][:],
                                             start=False, stop=True),
               key=f"p_n1_{s}")
            silu(a1[s][:], psm[1][0:H, 0:N], nu1_bc[:], (H, N), 0, f"k_a1_{s}",
                 [("pe", f"p_n1_{s}")])
            op("pe", lambda e, s=s: e.matmul(psm[0][0:H, 0:N], nu2_w[:], a1[s][:],
                                             start=True, stop=True),
               deps=[(SILU_ENG, f"k_a1_{s}")], key=f"p_n2_{s}")
            silu(a2[s][:], psm[0][0:H, 0:N], nu2_bc[:], (H, N), 1, f"k_a2_{s}",
                 [("pe", f"p_n2_{s}")])
            op("pe", lambda e, s=s: e.matmul(psm[1][0:H, 0:N], nu3_w[:], a2[s][:],
                                             start=True, stop=True),
               deps=[(SILU_ENG, f"k_a2_{s}")], key=f"p_n3_{s}")
            op("act", lambda e, s=s: e.activation(out=dh[s][:], in_=psm[1][0:H, 0:N],
                                                  func=AF.Identity, bias=nu3_bc[:]),
               deps=[("pe", f"p_n3_{s}")], key=f"k_dh_{s}")
            op("dve", lambda e, s=s: e.tensor_add(out=hnew[s][:], in0=dh[s][:],
                                                  in1=h_vT[s][:]),
               deps=[("act", f"k_dh_{s}")], key=f"k_hn_{s}")
            op("pe", lambda e, s=s: e.matmul(psm[0][0:D, 0:N], dx_w[:], hnew[s][:],
                                             start=True, stop=True),
               deps=[("dve", f"k_hn_{s}")], key=f"p_dx_{s}")
            op("act", lambda e, s=s: e.activation(out=dxT[s][:], in_=psm[0][0:D, 0:N],
                                                  func=AF.Tanh, bias=dx_bc[:]),
               deps=[("pe", f"p_dx_{s}")], key=f"k_dxT_{s}")
            op("dve", lambda e, s=s: e.reduce_sum(out=mu[s][:], in_=dxT[s][:],
                                                  axis=mybir.AxisListType.X),
               deps=[("act", f"k_dxT_{s}")], key=f"k_mu_{s}")
            op("act", lambda e, s=s: e.mul(out=mus[s][:], in_=mu[s][:], mul=1.0 / N),
               deps=[("dve", f"k_mu_{s}"), ("act", "k_sp3")], key=f"k_mus_{s}")
            op("dve", lambda e, s=s: e.tensor_tensor(
                out=dxo[s][:], in0=dxT[s][:], in1=mus[s][:].to_broadcast((D, N)),
                op=ALU.subtract), deps=[("act", f"k_mus_{s}")], key=f"k_dxs_{s}")
            op("dve", lambda e, s=s: e.tensor_tensor(
                out=dxo[s][:], in0=dxo[s][:], in1=sp3[:].to_broadcast((D, N)),
                op=ALU.mult), deps=[("dve", f"k_dxs_{s}")], key=f"k_dxo_{s}")
            op("sync", lambda e, s=s: e.dma_start(out=out_ext[s], in_=dxo[s][:]),
               deps=[("dve", f"k_dxo_{s}")], key=f"d_out_{s}", sem="dma_o")

        # ---- phase A: assign cumulative marks ----
        SEMS = ("dma_w", "dma_x", "dma_o", "pe", "act", "dve")
        counts = {sn: 0 for sn in SEMS}
        marks = {}

        def op_sem(entry):
            eng, emit, deps, key, sem = entry
            if sem is not None:
                return sem
            return {"pe": "pe", "act": "act", "dve": "dve"}[eng]

        def op_amt(sem):
            return 16 if sem.startswith("dma") else 1

        for entry in OPS:
            sem = op_sem(entry)
            counts[sem] += op_amt(sem)
            if entry[3] is not None:
                marks[(sem, entry[3])] = counts[sem]
        for sn in SEMS:
            marks[(sn, "TOTAL")] = counts[sn]

        # ---- phase B: emit per-engine programs ----
        with ExitStack() as sctx:
            sems = {sname: sctx.enter_context(nc.semaphore(f"{sname}_sem"))
                    for sname in SEMS}
            block = sctx.enter_context(nc.Block())

            def emit_engine(eng_name, eng):
                waited = {sn: 0 for sn in SEMS}
                for entry in OPS:
                    oeng, emit, deps, key, semov = entry
                    if oeng != eng_name:
                        continue
                    own = op_sem(entry)
                    for (sname, dkey) in deps:
                        val = marks[(sname, dkey)]
                        if val > waited[sname]:
                            eng.wait_ge(sems[sname], val)
                            waited[sname] = val
                    instr = emit(eng)
                    instr.then_inc(sems[own], op_amt(own))

            @block.sync
            def _(eng):
                emit_engine("sync", eng)

            @block.tensor
            def _(eng):
                emit_engine("pe", eng)

            @block.scalar
            def _(eng):
                emit_engine("act", eng)

            @block.vector
            def _(eng):
                emit_engine("dve", eng)

    return nc


def _prep_inputs(x, spin, ne_w, ne_b, ee1_w, ee1_b, ee2_w, ee2_b, rve_w, rev_w,
                 eu1_w, eu1_b, eu2_w, eu2_b, nu1_w, nu1_b, nu2_w, nu2_b,
                 nu3_w, nu3_b, dx_w, dx_b, bf_raw):
    f32 = np.float32
    bf = ml_dtypes.bfloat16
    x = np.asarray(x, f32)
    spin_f = np.asarray(spin, f32)

    ee1 = np.asarray(ee1_w, f32)
    sel = np.zeros((IBLK, 128, H), f32)
    for a in range(IBLK):
        sel[a, 3 * a:3 * a + 3, :] = -ee1[0:3, :]
        sel[a, 64 + a, :] = ee1[3, :]
        sel[a, 96 + a, :] = ee1[4, :]

    eu1 = np.asarray(eu1_w, f32).reshape(3, H, H)
    shared = {
        "sel": sel.astype(bf),
        "ne_w": np.asarray(ne_w, f32), "ne_b": np.asarray(ne_b, f32).reshape(H, 1),
        "ee1_b": np.asarray(ee1_b, f32).reshape(H, 1),
        "ee2_b": np.asarray(ee2_b, f32).reshape(H, 1),
        "rve_w": np.asarray(rve_w, f32), "rev_w": np.asarray(rev_w, f32),
        "eu1_w_f": eu1,
        "eu1_b": np.asarray(eu1_b, f32).reshape(H, 1),
        "eu2_b": np.asarray(eu2_b, f32).reshape(H, 1),
        "nu1_w": np.asarray(nu1_w, f32).reshape(2, H, H),
        "nu1_b": np.asarray(nu1_b, f32).reshape(H, 1),
        "nu2_w": np.asarray(nu2_w, f32),
        "nu2_b": np.asarray(nu2_b, f32).reshape(H, 1),
        "nu3_w": np.asarray(nu3_w, f32),
        "nu3_b": np.asarray(nu3_b, f32).reshape(H, 1),
        "dx_w": np.asarray(dx_w, f32), "dx_b": np.asarray(dx_b, f32).reshape(D, 1),
        "bf_raw": np.asarray(bf_raw, f32).reshape(1, 1),
        "ee2_w": np.asarray(ee2_w, f32).astype(bf),
        "U1b": eu1[0].astype(bf),
        "eu2_w": np.asarray(eu2_w, f32).astype(bf),
        "identb": np.eye(128, dtype=f32).astype(bf),
    }

    in_maps = []
    for c in range(NCORES):
        xs = x[c * BPC:(c + 1) * BPC]
        xT = np.ascontiguousarray(xs.transpose(0, 2, 1))
        xt4 = np.concatenate(
            [xT, np.broadcast_to(spin_f[None, None, :], (BPC, 1, N))], axis=1
        )
        m = dict(shared)
        m["xt4"] = np.ascontiguousarray(xt4)
        m["xtrep"] = np.ascontiguousarray(np.tile(xT, (1, IBLK, 1)))
        m["xiall"] = np.ascontiguousarray(
            xs.reshape(BPC, NBLK, IBLK * D).transpose(0, 2, 1))
        in_maps.append(m)
    return in_maps


def kernel(**inputs):
    from concourse.bass_utils import run_bass_kernel_spmd

    if "nc" not in _CACHE:
        _CACHE["nc"] = _build_nc()
    nc = _CACHE["nc"]

    in_maps = _prep_inputs(**inputs)
    res = run_bass_kernel_spmd(nc, in_maps, core_ids=list(range(NCORES)))
    outs = [np.asarray(r["out"]).reshape(BPC, D, N) for r in res.results]
    full = np.concatenate(outs, axis=0)
    return np.ascontiguousarray(full.transpose(0, 2, 1)).astype(np.float32)



# revision 15
# speedup vs baseline: 1.6918x; 1.6918x over previous
"""Trainium2 Bass kernel for CTNNBackflowNet forward (gnn_message_passing).

B=16, N=128, D=3, H=128.  Data-parallel: 2 samples/core x 8 NeuronCores.
Raw Bass Block style with explicit semaphores (standalone wait_ge).

v2 redesign (vs the group-serial baseline):
  * Host packs per-sample edge features E[5, N*N] = [x_i-x_j | r1 | r2]
    (pure functions of the input x), so edge layer-1 is ONE K=5 bf16
    matmul per 4-electron group.  No device sqrt -> Act engine runs only
    Silu/Identity/Tanh (one activation table, zero reloads).
  * ee2 folded into eu1:  W1C = ee2_w @ eu1_w[:H] (host).  The per-group
    ee2 matmul and PSUM->SBUF copy disappear.
  * j-sum moved BEFORE eu2 (sum and linear map commute); eu2 and rev_w
    fuse into one per-sample fp32 matmul W2R = eu2_w @ rev_w with bias
    mvc = 127 * rev_w^T eu2_b.
  * softplus(bf_raw) computed on host (scalar input transform).
  * Emission order is software-pipelined: each engine's in-order program
    interleaves stages of different groups (stage skew), 3-deep PSUM and
    SBUF rotation, so PE/Act/DVE overlap instead of ping-ponging.
Pipeline per group g (4 electrons, 512 edge-columns):
  slot g   PE  : pre1(g)            -> ppre1[g%3]
  slot g   DVE : tc(g)=t3_j+C2_i    -> tc_t[g%3] (bf16)
  slot g+1 Act : silu he1(g)        -> he1_t[g%3] (bf16)
  slot g+2 PE  : W1C@he1 + I@tc     -> ppeu[g%3]
  slot g+3 Act : silu heu1(g)       -> heu1_t[g%3] (bf16)
  slot g+4 DVE : j-reduce, diag-sub -> Hsum[s][:, 4g:4g+4] (f32)
Per-sample prep (h_v, vie, t3b, C2b) and tail (W2R, node MLP, dx head)
are injected into free slots; sample-0 tail overlaps sample-1 groups.
"""

import numpy as np
import ml_dtypes

B, N, D = 16, 128, 3
H = 128
EPS = 1e-12
NCORES = 8
BPC = B // NCORES
GRP = 4
GPS = N // GRP          # groups per sample (32)
NG = BPC * GPS          # group slots per core (64)

_CACHE = {}
SIM_COMPAT = False  # decompose Silu (CoreSim lacks it); flips silu keys to dve


def _build_nc():
    import concourse.bass as bass
    import concourse.mybir as mybir
    from contextlib import ExitStack

    f32 = mybir.dt.float32
    bf16 = mybir.dt.bfloat16
    AF = mybir.ActivationFunctionType
    ALU = mybir.AluOpType

    nc = bass.Bass()
    P = {}

    def par(name, shape, dt=f32):
        P[name] = nc.declare_dram_parameter(name, list(shape), dt, isOutput=False)
        return P[name]

    par("E", (BPC, 5, N * N), bf16)
    par("xt4", (BPC, 4, N))
    par("ee1bw", (5, H), bf16)
    par("W1C", (H, H), bf16)
    par("identb", (H, H), bf16)
    par("ee1_b", (H, 1)); par("euc", (H, 1))
    par("ne_w", (4, H)); par("ne_b", (H, 1))
    par("rve_w", (H, H))
    par("U3", (H, H)); par("U2", (H, H))
    par("W2R", (H, H)); par("mvc", (H, 1))
    par("nu1a", (H, H)); par("nu1b", (H, H)); par("nu1_b", (H, 1))
    par("nu2_w", (H, H)); par("nu2_b", (H, 1))
    par("nu3_w", (H, H)); par("nu3_b", (H, 1))
    par("dx_w", (H, D)); par("dx_b", (D, 1))
    par("spv", (D, 2))
    out_ext = nc.declare_dram_parameter("out", [BPC, D, N], f32, isOutput=True)

    ctx = ExitStack()

    def sb(name, shape, dt=f32):
        return ctx.enter_context(nc.sbuf_tensor('s_' + name, list(shape), dt))

    def ps(name, shape):
        return ctx.enter_context(nc.psum_tensor('ps_' + name, list(shape), f32))

    with ctx:
        E_sb = [sb(f"E_{s}", (5, N * N), bf16) for s in range(BPC)]
        xt4 = [sb(f"xt4_{s}", (4, N)) for s in range(BPC)]
        ee1bw = sb("ee1bw", (5, H), bf16)
        W1C = sb("W1C", (H, H), bf16)
        identb = sb("identb", (H, H), bf16)
        ee1_b = sb("ee1_b", (H, 1)); euc = sb("euc", (H, 1))
        ne_w = sb("ne_w", (4, H)); ne_b = sb("ne_b", (H, 1))
        rve_w = sb("rve_w", (H, H))
        U3 = sb("U3", (H, H)); U2 = sb("U2", (H, H))
        W2R = sb("W2R", (H, H)); mvc = sb("mvc", (H, 1))
        nu1a = sb("nu1a", (H, H)); nu1b = sb("nu1b", (H, H))
        nu1_bc = sb("nu1_bc", (H, 1))
        nu2_w = sb("nu2_w", (H, H)); nu2_bc = sb("nu2_bc", (H, 1))
        nu3_w = sb("nu3_w", (H, H)); nu3_bc = sb("nu3_bc", (H, 1))
        dx_w = sb("dx_w", (H, D)); dx_bc = sb("dx_bc", (D, 1))
        spv = sb("spv", (D, 2))

        he1_t = [sb(f"he1_{p}", (H, GRP * N), bf16) for p in range(3)]
        heu1_t = [sb(f"heu1_{p}", (H, GRP * N), bf16) for p in range(3)]
        tc_t = [sb(f"tc_{p}", (H, GRP * N), bf16) for p in range(3)]
        sums_t = [sb(f"sums_{p}", (H, GRP)) for p in range(2)]
        sg_t = ([sb(f"sg_{p}", (H, GRP * N)) for p in range(3)]
                if SIM_COMPAT else None)
        sg2_t = ([sb(f"sg2_{p}", (H, GRP * N)) for p in range(3)]
                 if SIM_COMPAT else None)
        sgn = sb("sgn", (H, N)) if SIM_COMPAT else None

        h_vT = [sb(f"h_vT_{s}", (H, N)) for s in range(BPC)]
        vieT = [sb(f"vieT_{s}", (H, N)) for s in range(BPC)]
        t3b = [sb(f"t3b_{s}", (H, N), bf16) for s in range(BPC)]
        C2b = [sb(f"C2b_{s}", (H, N), bf16) for s in range(BPC)]
        Hsum = [sb(f"Hsum_{s}", (H, N)) for s in range(BPC)]
        m_v = [sb(f"m_v_{s}", (H, N)) for s in range(BPC)]
        a1 = [sb(f"a1_{s}", (H, N)) for s in range(BPC)]
        a2 = [sb(f"a2_{s}", (H, N)) for s in range(BPC)]
        dh = [sb(f"dh_{s}", (H, N)) for s in range(BPC)]
        hnew = [sb(f"hnew_{s}", (H, N)) for s in range(BPC)]
        dxT = [sb(f"dxT_{s}", (D, N)) for s in range(BPC)]
        dxo = [sb(f"dxo_{s}", (D, N)) for s in range(BPC)]
        mu = [sb(f"mu_{s}", (D, 1)) for s in range(BPC)]
        mus = [sb(f"mus_{s}", (D, 1)) for s in range(BPC)]

        ppre1 = [ps(f"ppre1_{p}", (128, 512)) for p in range(3)]
        ppeu = [ps(f"ppeu_{p}", (128, 512)) for p in range(3)]
        psm = [ps(f"psm_{p}", (128, 512)) for p in range(2)]

        OPS = []

        def op(engine, emit, deps=(), key=None, sem=None):
            OPS.append((engine, emit, list(deps), key, sem))

        def dma(dst, src, deps=(), key=None, cls="dma_w"):
            op("sync", lambda e, d=dst, s=src: e.dma_start(out=d, in_=s), deps,
               key, sem=cls)

        SILU_ENG = "dve" if SIM_COMPAT else "act"

        def silu_op(out_ap, in_ap, bias_ap, scratch_ap, key, deps):
            if not SIM_COMPAT:
                op("act", lambda e, o=out_ap, i=in_ap, b=bias_ap: e.activation(
                    out=o, in_=i, func=AF.Silu, bias=b), deps=deps, key=key)
            else:
                op("act", lambda e, o=scratch_ap, i=in_ap, b=bias_ap:
                   e.activation(out=o, in_=i, func=AF.Sigmoid, bias=b),
                   deps=deps)
                op("act", lambda e, o=out_ap, i=in_ap, b=bias_ap: e.activation(
                    out=o, in_=i, func=AF.Identity, bias=b), key=key + "_i")
                op("dve", lambda e, o=out_ap, sc=scratch_ap: e.tensor_mul(
                    out=o, in0=o, in1=sc),
                   deps=[("act", key + "_i")], key=key)

        # ---- input DMAs: E tiles first (pipeline head), then xt4, weights
        for s in range(BPC):
            dma(E_sb[s][:], P["E"][s], key=f"d_E_{s}", cls="dma_x")
        for s in range(BPC):
            dma(xt4[s][:], P["xt4"][s], key=f"d_xt4_{s}", cls="dma_x")
        for nm, dst in [("ee1bw", ee1bw), ("W1C", W1C), ("identb", identb),
                        ("ee1_b", ee1_b), ("euc", euc), ("ne_w", ne_w),
                        ("ne_b", ne_b), ("rve_w", rve_w), ("U3", U3),
                        ("U2", U2), ("W2R", W2R), ("mvc", mvc),
                        ("nu1a", nu1a), ("nu1b", nu1b), ("nu1_b", nu1_bc),
                        ("nu2_w", nu2_w), ("nu2_b", nu2_bc),
                        ("nu3_w", nu3_w), ("nu3_b", nu3_bc),
                        ("dx_w", dx_w), ("dx_b", dx_bc), ("spv", spv)]:
            dma(dst[:], P[nm][:])

        # ---------------- per-sample prep ----------------
        def emit_prep(s):
            psm_hv_dep = [("act", "k_t3_0")] if s == 1 else []
            psm_vie_dep = [("dve", "k_c2_0")] if s == 1 else []
            op("pe", lambda e, s=s: e.matmul(psm[0][0:H, 0:N], ne_w[:], xt4[s][:],
                                             start=True, stop=True),
               deps=[("dma_w", "TOTAL"), ("dma_x", "TOTAL")] + psm_hv_dep,
               key=f"p_hv_{s}")
            op("act", lambda e, s=s: e.activation(out=h_vT[s][:],
                                                  in_=psm[0][0:H, 0:N],
                                                  func=AF.Identity, bias=ne_b[:]),
               deps=[("pe", f"p_hv_{s}")], key=f"k_hv_{s}")
            op("pe", lambda e, s=s: e.matmul(psm[1][0:H, 0:N], rve_w[:], h_vT[s][:],
                                             start=True, stop=True),
               deps=[("act", f"k_hv_{s}")] + psm_vie_dep, key=f"p_vie_{s}")
            op("dve", lambda e, s=s: e.tensor_copy(out=vieT[s][:],
                                                   in_=psm[1][0:H, 0:N]),
               deps=[("pe", f"p_vie_{s}")], key=f"k_vie_{s}")
            op("pe", lambda e, s=s: e.matmul(psm[0][0:H, 0:N], U3[:], vieT[s][:],
                                             start=True, stop=True),
               deps=[("dve", f"k_vie_{s}"), ("act", f"k_hv_{s}")],
               key=f"p_t3_{s}")
            op("act", lambda e, s=s: e.activation(out=t3b[s][:],
                                                  in_=psm[0][0:H, 0:N],
                                                  func=AF.Copy),
               deps=[("pe", f"p_t3_{s}")], key=f"k_t3_{s}")
            op("pe", lambda e, s=s: e.matmul(psm[1][0:H, 0:N], U2[:], vieT[s][:],
                                             start=True, stop=True),
               deps=[("dve", f"k_vie_{s}")], key=f"p_c2_{s}")
            op("dve", lambda e, s=s: e.tensor_copy(out=C2b[s][:],
                                                   in_=psm[1][0:H, 0:N]),
               deps=[("pe", f"p_c2_{s}")], key=f"k_c2_{s}")

        # ---------------- per-sample tail (stage st = 0..7) ----------------
        def emit_tail(s, st):
            if st == 0:
                psm0_dep = [("act", "k_t3_1")] if s == 0 else [("act", "k_tanh_0")]
                op("pe", lambda e, s=s: e.matmul(psm[0][0:H, 0:N], W2R[:],
                                                 Hsum[s][:], start=True, stop=True),
                   deps=[("dve", f"k_diag_{s * GPS + GPS - 1}")] + psm0_dep,
                   key=f"p_mv_{s}")
                op("act", lambda e, s=s: e.activation(out=m_v[s][:],
                                                      in_=psm[0][0:H, 0:N],
                                                      func=AF.Identity, bias=mvc[:]),
                   deps=[("pe", f"p_mv_{s}")], key=f"k_mv_{s}")
            elif st == 1:
                psm1_dep = ([("dve", "k_c2_1")] if s == 0
                            else [("act", "k_dh_0")])
                op("pe", lambda e, s=s: e.matmul(psm[1][0:H, 0:N], nu1a[:],
                                                 h_vT[s][:], start=True, stop=False),
                   deps=[("act", f"k_mv_{s}")] + psm1_dep)
                op("pe", lambda e, s=s: e.matmul(psm[1][0:H, 0:N], nu1b[:],
                                                 m_v[s][:], start=False, stop=True),
                   key=f"p_n1_{s}")
                silu_op(a1[s][:], psm[1][0:H, 0:N], nu1_bc[:],
                        sgn[:] if SIM_COMPAT else None,
                        f"k_a1_{s}", [("pe", f"p_n1_{s}")])
            elif st == 2:
                op("pe", lambda e, s=s: e.matmul(psm[0][0:H, 0:N], nu2_w[:],
                                                 a1[s][:], start=True, stop=True),
                   deps=[(SILU_ENG, f"k_a1_{s}"), ("act", f"k_mv_{s}")],
                   key=f"p_n2_{s}")
                silu_op(a2[s][:], psm[0][0:H, 0:N], nu2_bc[:],
                        sgn[:] if SIM_COMPAT else None,
                        f"k_a2_{s}", [("pe", f"p_n2_{s}")])
            elif st == 3:
                op("pe", lambda e, s=s: e.matmul(psm[1][0:H, 0:N], nu3_w[:],
                                                 a2[s][:], start=True, stop=True),
                   deps=[(SILU_ENG, f"k_a2_{s}")], key=f"p_n3_{s}")
                op("act", lambda e, s=s: e.activation(out=dh[s][:],
                                                      in_=psm[1][0:H, 0:N],
                                                      func=AF.Identity,
                                                      bias=nu3_bc[:]),
                   deps=[("pe", f"p_n3_{s}")], key=f"k_dh_{s}")
            elif st == 4:
                op("dve", lambda e, s=s: e.tensor_add(out=hnew[s][:], in0=dh[s][:],
                                                      in1=h_vT[s][:]),
                   deps=[("act", f"k_dh_{s}")], key=f"k_hn_{s}")
            elif st == 5:
                op("pe", lambda e, s=s: e.matmul(psm[0][0:D, 0:N], dx_w[:],
                                                 hnew[s][:], start=True, stop=True),
                   deps=[("dve", f"k_hn_{s}"), (SILU_ENG, f"k_a2_{s}")],
                   key=f"p_dx_{s}")
                op("act", lambda e, s=s: e.activation(out=dxT[s][:],
                                                      in_=psm[0][0:D, 0:N],
                                                      func=AF.Tanh, bias=dx_bc[:]),
                   deps=[("pe", f"p_dx_{s}")], key=f"k_tanh_{s}")
            elif st == 6:
                op("dve", lambda e, s=s: e.reduce_sum(out=mu[s][:], in_=dxT[s][:],
                                                      axis=mybir.AxisListType.X),
                   deps=[("act", f"k_tanh_{s}")], key=f"k_mu_{s}")
                op("act", lambda e, s=s: e.activation(out=mus[s][:], in_=mu[s][:],
                                                      func=AF.Copy,
                                                      scale=spv[:, 1:2]),
                   deps=[("dve", f"k_mu_{s}")], key=f"k_mus_{s}")
            elif st == 7:
                op("dve", lambda e, s=s: e.tensor_tensor(
                    out=dxo[s][:], in0=dxT[s][:],
                    in1=spv[:, 0:1].to_broadcast((D, N)), op=ALU.mult),
                   deps=[("act", f"k_mus_{s}")], key=f"k_dxs_{s}")
                op("dve", lambda e, s=s: e.tensor_tensor(
                    out=dxo[s][:], in0=dxo[s][:],
                    in1=mus[s][:].to_broadcast((D, N)), op=ALU.subtract),
                   deps=[("dve", f"k_dxs_{s}")], key=f"k_dxo_{s}")
                op("sync", lambda e, s=s: e.dma_start(out=out_ext[s],
                                                      in_=dxo[s][:]),
                   deps=[("dve", f"k_dxo_{s}")], key=f"d_out_{s}", sem="dma_o")

        # ---------------- pipelined group slots ----------------
        emit_prep(0)

        PREP1_SLOT = 26          # prep for sample 1 spans emission slot 26
        TAIL_BASE = [38, NG + 5]  # tail stage st of sample s at base+st

        def group_ops(slot):
            # stage 1: PE pre1
            if slot < NG:
                g = slot
                s = g // GPS
                off = (g % GPS) * GRP * N
                deps = [("dma_x", "TOTAL")]
                if g >= 3:
                    deps.append((SILU_ENG, f"k_he1_{g - 3}"))
                op("pe", lambda e, g=g, s=s, off=off: e.matmul(
                    ppre1[g % 3][0:H, 0:GRP * N], ee1bw[:],
                    E_sb[s][:, off:off + GRP * N], start=True, stop=True),
                   deps=deps, key=f"p_pre1_{g}")
            # stage 1b: DVE tc
            if slot < NG:
                g = slot
                s = g // GPS
                c0 = (g % GPS) * GRP
                deps = [("act", f"k_t3_{s}"), ("dve", f"k_c2_{s}")]
                if g >= 3:
                    deps.append(("pe", f"p_eu2_{g - 3}"))
                op("dve", lambda e, g=g, s=s, c0=c0: e.tensor_tensor(
                    out=tc_t[g % 3][:].rearrange("p (a j) -> p a j", j=N),
                    in0=t3b[s][:, None, :].to_broadcast((H, GRP, N)),
                    in1=C2b[s][:, c0:c0 + GRP, None].to_broadcast((H, GRP, N)),
                    op=ALU.add),
                   deps=deps, key=f"k_tc_{g}")
            # stage 2: Act silu he1
            if 0 <= slot - 1 < NG:
                g = slot - 1
                deps = [("pe", f"p_pre1_{g}")]
                if g >= 3:
                    deps.append(("pe", f"p_eu_{g - 3}"))
                silu_op(he1_t[g % 3][:], ppre1[g % 3][0:H, 0:GRP * N],
                        ee1_b[:], sg_t[g % 3][:] if SIM_COMPAT else None,
                        f"k_he1_{g}", deps)
            # stage 3: PE W1C + ident(tc)
            if 0 <= slot - 2 < NG:
                g = slot - 2
                deps = [(SILU_ENG, f"k_he1_{g}"), ("dve", f"k_tc_{g}")]
                if g >= 3:
                    deps.append((SILU_ENG, f"k_heu1_{g - 3}"))
                op("pe", lambda e, g=g: e.matmul(
                    ppeu[g % 3][0:H, 0:GRP * N], W1C[:], he1_t[g % 3][:],
                    start=True, stop=False),
                   deps=deps, key=f"p_eu_{g}")
                op("pe", lambda e, g=g: e.matmul(
                    ppeu[g % 3][0:H, 0:GRP * N], identb[:], tc_t[g % 3][:],
                    start=False, stop=True),
                   key=f"p_eu2_{g}")
            # stage 4: Act silu heu1
            if 0 <= slot - 3 < NG:
                g = slot - 3
                deps = [("pe", f"p_eu2_{g}")]
                if g >= 3:
                    deps.append(("dve", f"k_diag_{g - 3}"))
                silu_op(heu1_t[g % 3][:], ppeu[g % 3][0:H, 0:GRP * N],
                        euc[:], sg2_t[g % 3][:] if SIM_COMPAT else None,
                        f"k_heu1_{g}", deps)
            # stage 5: DVE reduce + diag-sub
            if 0 <= slot - 4 < NG:
                g = slot - 4
                s = g // GPS
                c0 = (g % GPS) * GRP
                op("dve", lambda e, g=g: e.reduce_sum(
                    out=sums_t[g % 2][:],
                    in_=heu1_t[g % 3][:].rearrange("p (a j) -> p a j", j=N),
                    axis=mybir.AxisListType.X),
                   deps=[(SILU_ENG, f"k_heu1_{g}")], key=f"k_red_{g}")
                op("dve", lambda e, g=g, s=s, c0=c0: e.tensor_tensor(
                    out=Hsum[s][:, c0:c0 + GRP], in0=sums_t[g % 2][:],
                    in1=heu1_t[g % 3][:, c0:c0 + (GRP - 1) * (N + 1) + 1:N + 1],
                    op=ALU.subtract),
                   deps=[("dve", f"k_red_{g}")], key=f"k_diag_{g}")

        for slot in range(NG + 5):
            group_ops(slot)
            if slot == PREP1_SLOT:
                emit_prep(1)
            for s in range(BPC):
                st = slot - TAIL_BASE[s]
                if 0 <= st < 8:
                    emit_tail(s, st)
        for st in range(8):
            if NG + 5 - TAIL_BASE[1] <= st < 8:
                emit_tail(1, st)

        # ---- phase A: assign cumulative marks ----
        SEMS = ("dma_w", "dma_x", "dma_o", "pe", "act", "dve")
        counts = {sn: 0 for sn in SEMS}
        marks = {}

        def op_sem(entry):
            eng, emit, deps, key, sem = entry
            if sem is not None:
                return sem
            return {"pe": "pe", "act": "act", "dve": "dve"}[eng]

        def op_amt(sem):
            return 16 if sem.startswith("dma") else 1

        for entry in OPS:
            sem = op_sem(entry)
            counts[sem] += op_amt(sem)
            if entry[3] is not None:
                marks[(sem, entry[3])] = counts[sem]
        for sn in SEMS:
            marks[(sn, "TOTAL")] = counts[sn]

        # ---- phase B: emit per-engine programs ----
        from contextlib import ExitStack as ES2
        with ES2() as sctx:
            sems = {sname: sctx.enter_context(nc.semaphore(f"{sname}_sem"))
                    for sname in SEMS}
            block = sctx.enter_context(nc.Block())

            def emit_engine(eng_name, eng):
                waited = {sn: 0 for sn in SEMS}
                for entry in OPS:
                    oeng, emit, deps, key, semov = entry
                    if oeng != eng_name:
                        continue
                    own = op_sem(entry)
                    for (sname, dkey) in deps:
                        val = marks[(sname, dkey)]
                        if val > waited[sname]:
                            eng.wait_ge(sems[sname], val)
                            waited[sname] = val
                    instr = emit(eng)
                    instr.then_inc(sems[own], op_amt(own))

            @block.sync
            def _(eng):
                emit_engine("sync", eng)

            @block.tensor
            def _(eng):
                emit_engine("pe", eng)

            @block.scalar
            def _(eng):
                emit_engine("act", eng)

            @block.vector
            def _(eng):
                emit_engine("dve", eng)

    return nc


def _prep_inputs(x, spin, ne_w, ne_b, ee1_w, ee1_b, ee2_w, ee2_b, rve_w, rev_w,
                 eu1_w, eu1_b, eu2_w, eu2_b, nu1_w, nu1_b, nu2_w, nu2_b,
                 nu3_w, nu3_b, dx_w, dx_b, bf_raw):
    f32 = np.float32
    bf = ml_dtypes.bfloat16
    x = np.asarray(x, f32)
    spin_f = np.asarray(spin, f32)

    eu1 = np.asarray(eu1_w, f32)
    U1, U2, U3 = eu1[0:H], eu1[H:2 * H], eu1[2 * H:3 * H]
    ee2 = np.asarray(ee2_w, f32)
    rev = np.asarray(rev_w, f32)
    eu2 = np.asarray(eu2_w, f32)
    sp = float(np.log1p(np.exp(np.float64(np.asarray(bf_raw)))))

    shared = {
        "ee1bw": np.asarray(ee1_w, f32).astype(bf),
        "W1C": (ee2 @ U1).astype(bf),
        "identb": np.eye(H, dtype=f32).astype(bf),
        "ee1_b": np.asarray(ee1_b, f32).reshape(H, 1),
        "euc": (U1.T @ np.asarray(ee2_b, f32) + np.asarray(eu1_b, f32))
               .reshape(H, 1).astype(f32),
        "ne_w": np.asarray(ne_w, f32),
        "ne_b": np.asarray(ne_b, f32).reshape(H, 1),
        "rve_w": np.asarray(rve_w, f32),
        "U3": np.ascontiguousarray(U3), "U2": np.ascontiguousarray(U2),
        "W2R": (eu2 @ rev).astype(f32),
        "mvc": ((N - 1.0) * (rev.T @ np.asarray(eu2_b, f32))).reshape(H, 1),
        "nu1a": np.ascontiguousarray(np.asarray(nu1_w, f32)[0:H]),
        "nu1b": np.ascontiguousarray(np.asarray(nu1_w, f32)[H:2 * H]),
        "nu1_b": np.asarray(nu1_b, f32).reshape(H, 1),
        "nu2_w": np.asarray(nu2_w, f32),
        "nu2_b": np.asarray(nu2_b, f32).reshape(H, 1),
        "nu3_w": np.asarray(nu3_w, f32),
        "nu3_b": np.asarray(nu3_b, f32).reshape(H, 1),
        "dx_w": np.asarray(dx_w, f32),
        "dx_b": np.asarray(dx_b, f32).reshape(D, 1),
        "spv": np.broadcast_to(np.array([sp, sp / N], f32), (D, 2)).copy(),
    }

    in_maps = []
    for c in range(NCORES):
        xs = x[c * BPC:(c + 1) * BPC]          # (BPC, N, D)
        E = np.empty((BPC, 5, N * N), f32)
        for s in range(BPC):
            diff = xs[s][:, None, :] - xs[s][None, :, :]   # (i, j, d) = x_i - x_j
            r2 = np.sum(diff * diff, axis=-1)
            r1 = np.sqrt(r2 + EPS)
            E[s, 0:3] = diff.transpose(2, 0, 1).reshape(3, N * N)
            E[s, 3] = r1.reshape(N * N)
            E[s, 4] = r2.reshape(N * N)
        xT = np.ascontiguousarray(xs.transpose(0, 2, 1))   # (BPC, D, N)
        xt4 = np.concatenate(
            [xT, np.broadcast_to(spin_f[None, None, :], (BPC, 1, N))], axis=1)
        m = dict(shared)
        m["E"] = E.astype(bf)
        m["xt4"] = np.ascontiguousarray(xt4)
        in_maps.append(m)
    return in_maps


def kernel(**inputs):
    from concourse.bass_utils import run_bass_kernel_spmd

    if "nc" not in _CACHE:
        _CACHE["nc"] = _build_nc()
    nc = _CACHE["nc"]

    in_maps = _prep_inputs(**inputs)
    res = run_bass_kernel_spmd(nc, in_maps, core_ids=list(range(NCORES)))
    outs = [np.asarray(r["out"]).reshape(BPC, D, N) for r in res.results]
    full = np.concatenate(outs, axis=0)
    return np.ascontiguousarray(full.transpose(0, 2, 1)).astype(np.float32)


# revision 30
# speedup vs baseline: 1.9168x; 1.1330x over previous
"""Trainium2 Bass kernel for CTNNBackflowNet forward (gnn_message_passing).

B=16, N=128, D=3, H=128.  Data-parallel: 2 samples/core x 8 NeuronCores.
Raw Bass Block style with explicit semaphores (standalone wait_ge).

Structure (all derived weights folded on host):
  * Host packs per-sample edge features E[5, N*N] = [x_i-x_j | r1 | r2]
    (pure functions of the input x), so edge layer-1 is one K=5 bf16
    matmul per 512-col PSUM bank half.  No device sqrt -> the Act engine
    runs only Silu/Identity/Tanh (one table, zero reloads).
  * ee2 folded into eu1:  W1C = ee2_w @ eu1_w[:H].
  * j-sum moved BEFORE eu2 (sum and linear map commute); eu2, rev_w and
    nu1's m_v half fuse into one matmul W2RN = eu2 @ rev @ nu1b applied
    to the aggregated Hsum; nu3 and dx_head fuse into W3D = nu3 @ dx_w.
  * Node prep collapses to three K=4 matmuls from xt4 = [x;spin] with
    host-folded (ne @ rve @ U3) / (ne @ rve @ U2) chains.
  * softplus(bf_raw) is a host scalar, baked as immediates (nc cached
    per value).
Pipeline per group g (8 electrons, 1024 cols over two PSUM banks):
  slot g   PE  : pre1(g) 2 halves    -> ppre1 (single-buffered)
  slot g+1 Act : silu he1(g)         -> he1_t[g%3] (bf16)
  slot g+2 PE  : (W1C@he1 + I@t3_j + I@C2_i) x2 halves -> ppeu[g%2]
  slot g+3 Act : silu heu1(g)        -> heu1_t[g%3] (bf16)
  slot g+4 DVE : j-reduce, diag-sub  -> Hsum[s][:, 8g:8g+8] (f32)
Act is the bottleneck engine (2 x 1038ns silus per slot, saturated).
Per-sample prep/tail stages are split into a PE part and an Act/DVE part
emitted one slot later so the saturated in-order streams never block.
"""

import numpy as np
import ml_dtypes

B, N, D = 16, 128, 3
H = 128
EPS = 1e-12
NCORES = 8
BPC = B // NCORES
GRP = 8
GPS = N // GRP          # groups per sample (16)
NG = BPC * GPS          # group slots per core (32)

_CACHE = {}
SIM_COMPAT = False  # decompose Silu (CoreSim lacks it); flips silu keys to dve


def _build_nc(sp):
    import concourse.bass as bass
    import concourse.mybir as mybir
    from contextlib import ExitStack

    f32 = mybir.dt.float32
    bf16 = mybir.dt.bfloat16
    AF = mybir.ActivationFunctionType
    ALU = mybir.AluOpType

    nc = bass.Bass()
    P = {}

    def par(name, shape, dt=f32):
        P[name] = nc.declare_dram_parameter(name, list(shape), dt, isOutput=False)
        return P[name]

    par("E", (BPC, 5, N * N), bf16)
    par("xt4", (4, 2 * N))
    par("ne3", (4, 3 * H))            # [ne_w | ne@rve@U3 | ne@rve@U2]
    par("wpv", (H, 8))                # bias vectors (see host packing)
    par("wpbf", (H, 3 * H), bf16)     # [W1C | identb | ee1(pad to 128 rows)]
    par("wp32b", (H, 390))            # [W2RN | nu1a | nu2_w | W3D | dx_w]
    out_ext = nc.declare_dram_parameter("out", [BPC, D, N], f32, isOutput=True)

    ctx = ExitStack()

    def sb(name, shape, dt=f32):
        return ctx.enter_context(nc.sbuf_tensor('s_' + name, list(shape), dt))

    def ps(name, shape):
        return ctx.enter_context(nc.psum_tensor('ps_' + name, list(shape), f32))

    with ctx:
        E_sb = [sb(f"E_{s}", (5, N * N), bf16) for s in range(BPC)]
        xt4b = sb("xt4b", (4, 2 * N))
        xt4 = [xt4b[:, N * s:N * s + N] for s in range(BPC)]
        ne3 = sb("ne3", (4, 3 * H))
        ne_w = ne3[:, 0:H]; Wt3 = ne3[:, H:2 * H]; Wc2 = ne3[:, 2 * H:3 * H]
        wpv = sb("wpv", (H, 8))
        ne_b = wpv[:, 0:1]; c_t3 = wpv[:, 1:2]; c_c2 = wpv[:, 2:3]
        b1c = wpv[:, 3:4]; nu2_bc = wpv[:, 4:5]
        bdxc = wpv[0:D, 5:6]; ee1_b = wpv[:, 6:7]; euc = wpv[:, 7:8]
        wpbf = sb("wpbf", (H, 3 * H), bf16)
        W1C = wpbf[:, 0:H]; identb = wpbf[:, H:2 * H]
        ee1bw = wpbf[0:5, 2 * H:3 * H]
        wp32b = sb("wp32b", (H, 390))
        W2RN = wp32b[:, 0:128]; nu1a = wp32b[:, 128:256]
        nu2_w = wp32b[:, 256:384]
        W3D = wp32b[:, 384:387]; dx_w = wp32b[:, 387:390]

        he1_t = [sb(f"he1_{p}", (H, GRP * N), bf16) for p in range(3)]
        heu1_t = [sb(f"heu1_{p}", (H, GRP * N), bf16) for p in range(3)]
        sums_t = [sb(f"sums_{p}", (H, GRP)) for p in range(2)]
        sg_t = ([sb(f"sg_{p}", (H, GRP * N)) for p in range(3)]
                if SIM_COMPAT else None)
        sg2_t = ([sb(f"sg2_{p}", (H, GRP * N)) for p in range(3)]
                 if SIM_COMPAT else None)
        sgn = ([sb(f"sgn_{s}", (H, N)) for s in range(BPC)]
               if SIM_COMPAT else None)

        h_vT = [sb(f"h_vT_{s}", (H, N)) for s in range(BPC)]
        t3b = [sb(f"t3b_{s}", (H, N), bf16) for s in range(BPC)]
        C2b = [sb(f"C2b_{s}", (H, N), bf16) for s in range(BPC)]
        Hsum = [sb(f"Hsum_{s}", (H, N)) for s in range(BPC)]
        a1 = [sb(f"a1_{s}", (H, N)) for s in range(BPC)]
        a2 = [sb(f"a2_{s}", (H, N)) for s in range(BPC)]
        dxT = [sb(f"dxT_{s}", (D, N)) for s in range(BPC)]
        dxo = [sb(f"dxo_{s}", (D, N)) for s in range(BPC)]
        mu = [sb(f"mu_{s}", (D, 1)) for s in range(BPC)]
        mus = [sb(f"mus_{s}", (D, 1)) for s in range(BPC)]

        ppre1 = ps("ppre1", (128, GRP * N))
        ppeu = [ps(f"ppeu_{p}", (128, GRP * N)) for p in range(2)]
        psm = [ps(f"psm_{p}", (128, 512)) for p in range(2)]

        OPS = []

        def op(engine, emit, deps=(), key=None, sem=None):
            OPS.append((engine, emit, list(deps), key, sem))

        def dma(dst, src, deps=(), key=None, cls="dma_w"):
            op("sync", lambda e, d=dst, s=src: e.dma_start(out=d, in_=s), deps,
               key, sem=cls)

        def dma_p(dst, src, deps=(), key=None, cls="dma_w"):
            op("pool", lambda e, d=dst, s=src: e.dma_start(out=d, in_=s), deps,
               key, sem=cls)

        SILU_ENG = "dve" if SIM_COMPAT else "act"

        def silu_op(out_ap, in_ap, bias_ap, scratch_ap, key, deps):
            if not SIM_COMPAT:
                op("act", lambda e, o=out_ap, i=in_ap, b=bias_ap: e.activation(
                    out=o, in_=i, func=AF.Silu, bias=b), deps=deps, key=key)
            else:
                op("act", lambda e, o=scratch_ap, i=in_ap, b=bias_ap:
                   e.activation(out=o, in_=i, func=AF.Sigmoid, bias=b),
                   deps=deps)
                op("act", lambda e, o=out_ap, i=in_ap, b=bias_ap: e.activation(
                    out=o, in_=i, func=AF.Identity, bias=b), key=key + "_i")
                op("dve", lambda e, o=out_ap, sc=scratch_ap: e.tensor_mul(
                    out=o, in0=o, in1=sc),
                   deps=[("act", key + "_i")], key=key)

        # ---- input DMAs (sync queue = start-critical, pool queue = rest)
        dma(ne3[:, :], P["ne3"][:])
        dma(xt4b[:], P["xt4"][:], cls="dma_x")
        dma(wpv[:, :], P["wpv"][:])
        dma(wpbf[:], P["wpbf"][:], cls="dma_e0")
        dma(E_sb[0][:], P["E"][0], cls="dma_e0")
        dma_p(wp32b[:, :], P["wp32b"][:], cls="dma_wb")
        dma_p(E_sb[1][:], P["E"][1], cls="dma_e1")

        # -------- per-sample prep (stage st = 0..2, part pe|other) --------
        # psm[0] regions: [0:128] hv, [128:256] t3, [256:384] c2
        def emit_prep(s, st, part):
            if st == 0 and part == "pe":
                deps = [("dma_w", "TOTAL"), ("dma_x", "TOTAL")]
                if s == 1:
                    deps.append(("act", "k_hv_0"))
                op("pe", lambda e, s=s: e.matmul(psm[0][0:H, 0:N], ne_w,
                                                 xt4[s], start=True, stop=True),
                   deps=deps, key=f"p_hv_{s}")
            elif st == 0:
                op("act", lambda e, s=s: e.activation(out=h_vT[s][:],
                                                      in_=psm[0][0:H, 0:N],
                                                      func=AF.Identity,
                                                      bias=ne_b),
                   deps=[("pe", f"p_hv_{s}")], key=f"k_hv_{s}")
            elif st == 1 and part == "pe":
                deps = [("act", "k_t3_0")] if s == 1 else []
                op("pe", lambda e, s=s: e.matmul(psm[0][0:H, N:2 * N], Wt3,
                                                 xt4[s], start=True, stop=True),
                   deps=deps, key=f"p_t3_{s}")
            elif st == 1:
                op("act", lambda e, s=s: e.activation(out=t3b[s][:],
                                                      in_=psm[0][0:H, N:2 * N],
                                                      func=AF.Identity,
                                                      bias=c_t3),
                   deps=[("pe", f"p_t3_{s}")], key=f"k_t3_{s}")
            elif st == 2 and part == "pe":
                deps = [("dve", "k_c2_0")] if s == 1 else []
                op("pe", lambda e, s=s: e.matmul(psm[0][0:H, 2 * N:3 * N], Wc2,
                                                 xt4[s], start=True, stop=True),
                   deps=deps, key=f"p_c2_{s}")
            elif st == 2:
                op("dve", lambda e, s=s: e.tensor_tensor(
                    out=C2b[s][:], in0=psm[0][0:H, 2 * N:3 * N],
                    in1=c_c2.to_broadcast((H, N)), op=ALU.add),
                   deps=[("pe", f"p_c2_{s}")], key=f"k_c2_{s}")

        # -------- per-sample tail (stage st = 0..4, part pe|other) --------
        # psm[1] cols: [256s : 256s+128] n1, [256s+128 : 256s+256] n2
        # psm[0][0:D, 384:512]: dx accumulator (s0 then s1, tanh_0-ordered)
        def emit_tail(s, st, part):
            c1 = 256 * s
            if st == 0 and part == "pe":
                op("pe", lambda e, s=s, c1=c1: e.matmul(
                    psm[1][0:H, c1:c1 + N], nu1a, h_vT[s][:],
                    start=True, stop=False),
                   deps=[("act", f"k_hv_{s}"), ("dma_wb", "TOTAL")])
                dxa_deps = [("act", "k_tanh_0")] if s == 1 else []
                op("pe", lambda e, s=s: e.matmul(
                    psm[0][0:D, 384:512], dx_w, h_vT[s][:],
                    start=True, stop=False), deps=dxa_deps)
            elif st == 1 and part == "pe":
                op("pe", lambda e, s=s, c1=c1: e.matmul(
                    psm[1][0:H, c1:c1 + N], W2RN, Hsum[s][:],
                    start=False, stop=True),
                   deps=[("dve", f"k_diag_{s * GPS + GPS - 1}")],
                   key=f"p_n1_{s}")
            elif st == 1:
                silu_op(a1[s][:], psm[1][0:H, c1:c1 + N], b1c,
                        sgn[s][:] if SIM_COMPAT else None,
                        f"k_a1_{s}", [("pe", f"p_n1_{s}")])
            elif st == 2 and part == "pe":
                op("pe", lambda e, s=s, c1=c1: e.matmul(
                    psm[1][0:H, c1 + N:c1 + 2 * N], nu2_w, a1[s][:],
                    start=True, stop=True),
                   deps=[(SILU_ENG, f"k_a1_{s}")], key=f"p_n2_{s}")
            elif st == 2:
                silu_op(a2[s][:], psm[1][0:H, c1 + N:c1 + 2 * N], nu2_bc,
                        sgn[s][:] if SIM_COMPAT else None,
                        f"k_a2_{s}", [("pe", f"p_n2_{s}")])
            elif st == 3 and part == "pe":
                op("pe", lambda e, s=s: e.matmul(
                    psm[0][0:D, 384:512], W3D, a2[s][:],
                    start=False, stop=True),
                   deps=[(SILU_ENG, f"k_a2_{s}")], key=f"p_dx_{s}")
            elif st == 3:
                op("act", lambda e, s=s: e.activation(out=dxT[s][:],
                                                      in_=psm[0][0:D, 384:512],
                                                      func=AF.Tanh, bias=bdxc),
                   deps=[("pe", f"p_dx_{s}")], key=f"k_tanh_{s}")
            elif st == 4 and part != "pe":
                op("dve", lambda e, s=s: e.reduce_sum(out=mu[s][:],
                                                      in_=dxT[s][:],
                                                      axis=mybir.AxisListType.X),
                   deps=[("act", f"k_tanh_{s}")], key=f"k_mu_{s}")
                op("dve", lambda e, s=s: e.tensor_scalar(
                    out=mus[s][:], in0=mu[s][:], scalar1=float(sp) / N,
                    scalar2=None, op0=ALU.mult),
                   deps=[("dve", f"k_mu_{s}")], key=f"k_mus_{s}")
                op("dve", lambda e, s=s: e.scalar_tensor_tensor(
                    out=dxo[s][:], in0=dxT[s][:], scalar=float(sp),
                    in1=mus[s][:].to_broadcast((D, N)),
                    op0=ALU.mult, op1=ALU.subtract),
                   deps=[("dve", f"k_mus_{s}")], key=f"k_dxo_{s}")
                op("sync", lambda e, s=s: e.dma_start(out=out_ext[s],
                                                      in_=dxo[s][:]),
                   deps=[("dve", f"k_dxo_{s}")], key=f"d_out_{s}", sem="dma_o")

        # ---------------- pipelined group slots ----------------
        for st in range(3):
            emit_prep(0, st, "pe")
            emit_prep(0, st, "other")

        PREP1_BASE = 7   # prep(1): pe at 7+2*st, other at 8+2*st
        TAIL_BASE = [21, NG + 5]  # tail(s): pe at base+2*st, other at +1

        def group_ops(slot):
            # stage 1: PE pre1 (two 512-col bank halves; ppre1 single-buffered)
            if slot < NG:
                g = slot
                s = g // GPS
                off = (g % GPS) * GRP * N
                deps = [(f"dma_e{s}", "TOTAL")]
                if g >= 1:
                    deps.append((SILU_ENG, f"k_he1_{g - 1}"))
                op("pe", lambda e, s=s, off=off: e.matmul(
                    ppre1[0:H, 0:512], ee1bw,
                    E_sb[s][:, off:off + 512], start=True, stop=True),
                   deps=deps)
                op("pe", lambda e, s=s, off=off: e.matmul(
                    ppre1[0:H, 512:1024], ee1bw,
                    E_sb[s][:, off + 512:off + 1024], start=True, stop=True),
                   key=f"p_pre1_{g}")
            # stage 2: Act silu he1
            if 0 <= slot - 1 < NG:
                g = slot - 1
                deps = [("pe", f"p_pre1_{g}")]
                if g >= 3:
                    deps.append(("pe", f"p_eu2_{g - 3}"))
                silu_op(he1_t[g % 3][:], ppre1[0:H, 0:GRP * N],
                        ee1_b, sg_t[g % 3][:] if SIM_COMPAT else None,
                        f"k_he1_{g}", deps)
            # stage 3: PE W1C + ident injections (per 512-col bank half)
            if 0 <= slot - 2 < NG:
                g = slot - 2
                s = g // GPS
                c0 = (g % GPS) * GRP
                deps = [(SILU_ENG, f"k_he1_{g}"), ("act", f"k_t3_{s}"),
                        ("dve", f"k_c2_{s}")]
                if g >= 2:
                    deps.append((SILU_ENG, f"k_heu1_{g - 2}"))
                for h in range(2):
                    hb = 512 * h
                    op("pe", lambda e, g=g, hb=hb: e.matmul(
                        ppeu[g % 2][0:H, hb:hb + 512], W1C,
                        he1_t[g % 3][:, hb:hb + 512],
                        start=True, stop=False),
                       deps=(deps if h == 0 else ()),
                       key=(f"p_eu_{g}" if h == 0 else None))
                    op("pe", lambda e, g=g, s=s, hb=hb: e.matmul(
                        ppeu[g % 2][0:H, hb:hb + 512], identb,
                        t3b[s][:, None, :].to_broadcast((H, 4, N)),
                        start=False, stop=False))
                    op("pe", lambda e, g=g, s=s, c0=c0, h=h, hb=hb: e.matmul(
                        ppeu[g % 2][0:H, hb:hb + 512], identb,
                        C2b[s][:, c0 + 4 * h:c0 + 4 * h + 4,
                               None].to_broadcast((H, 4, N)),
                        start=False, stop=True),
                       key=(f"p_eu2_{g}" if h == 1 else None))
            # stage 4: Act silu heu1
            if 0 <= slot - 3 < NG:
                g = slot - 3
                deps = [("pe", f"p_eu2_{g}")]
                if g >= 3:
                    deps.append(("dve", f"k_diag_{g - 3}"))
                silu_op(heu1_t[g % 3][:], ppeu[g % 2][0:H, 0:GRP * N],
                        euc, sg2_t[g % 3][:] if SIM_COMPAT else None,
                        f"k_heu1_{g}", deps)
            # stage 5: DVE reduce + diag-sub
            if 0 <= slot - 4 < NG:
                g = slot - 4
                s = g // GPS
                c0 = (g % GPS) * GRP
                op("dve", lambda e, g=g: e.reduce_sum(
                    out=sums_t[g % 2][:],
                    in_=heu1_t[g % 3][:].rearrange("p (a j) -> p a j", j=N),
                    axis=mybir.AxisListType.X),
                   deps=[(SILU_ENG, f"k_heu1_{g}")], key=f"k_red_{g}")
                op("dve", lambda e, g=g, s=s, c0=c0: e.tensor_tensor(
                    out=Hsum[s][:, c0:c0 + GRP], in0=sums_t[g % 2][:],
                    in1=heu1_t[g % 3][:, c0:c0 + (GRP - 1) * (N + 1) + 1:N + 1],
                    op=ALU.subtract),
                   deps=[("dve", f"k_red_{g}")], key=f"k_diag_{g}")

        for slot in range(NG + 5):
            group_ops(slot)
            pst = slot - PREP1_BASE
            if 0 <= pst < 6:
                emit_prep(1, pst // 2, "pe" if pst % 2 == 0 else "other")
            for s in range(BPC):
                tst = slot - TAIL_BASE[s]
                if 0 <= tst < 10:
                    emit_tail(s, tst // 2, "pe" if tst % 2 == 0 else "other")
        for st in range(5):
            emit_tail(1, st, "pe")
            emit_tail(1, st, "other")

        # ---- phase A: assign cumulative marks ----
        SEMS = ("dma_w", "dma_x", "dma_e0", "dma_e1", "dma_wb",
                "dma_o", "pe", "act", "dve", "pool")
        counts = {sn: 0 for sn in SEMS}
        marks = {}

        def op_sem(entry):
            eng, emit, deps, key, sem = entry
            if sem is not None:
                return sem
            return {"pe": "pe", "act": "act", "dve": "dve",
                    "pool": "pool"}[eng]

        def op_amt(sem):
            return 16 if sem.startswith("dma") else 1

        for entry in OPS:
            sem = op_sem(entry)
            counts[sem] += op_amt(sem)
            if entry[3] is not None:
                marks[(sem, entry[3])] = counts[sem]
        for sn in SEMS:
            marks[(sn, "TOTAL")] = counts[sn]

        # ---- phase B: emit per-engine programs ----
        from contextlib import ExitStack as ES2
        with ES2() as sctx:
            sems = {sname: sctx.enter_context(nc.semaphore(f"{sname}_sem"))
                    for sname in SEMS}
            block = sctx.enter_context(nc.Block())

            def emit_engine(eng_name, eng):
                waited = {sn: 0 for sn in SEMS}
                for entry in OPS:
                    oeng, emit, deps, key, semov = entry
                    if oeng != eng_name:
                        continue
                    own = op_sem(entry)
                    for (sname, dkey) in deps:
                        val = marks[(sname, dkey)]
                        if val > waited[sname]:
                            eng.wait_ge(sems[sname], val)
                            waited[sname] = val
                    instr = emit(eng)
                    instr.then_inc(sems[own], op_amt(own))

            @block.sync
            def _(eng):
                emit_engine("sync", eng)

            @block.tensor
            def _(eng):
                emit_engine("pe", eng)

            @block.scalar
            def _(eng):
                emit_engine("act", eng)

            @block.vector
            def _(eng):
                emit_engine("dve", eng)

            @block.gpsimd
            def _(eng):
                emit_engine("pool", eng)

    return nc


def _prep_inputs(x, spin, ne_w, ne_b, ee1_w, ee1_b, ee2_w, ee2_b, rve_w, rev_w,
                 eu1_w, eu1_b, eu2_w, eu2_b, nu1_w, nu1_b, nu2_w, nu2_b,
                 nu3_w, nu3_b, dx_w, dx_b, bf_raw):
    f32 = np.float32
    bf = ml_dtypes.bfloat16
    x = np.asarray(x, f32)
    spin_f = np.asarray(spin, f32)

    eu1 = np.asarray(eu1_w, f32)
    U1, U2, U3 = eu1[0:H], eu1[H:2 * H], eu1[2 * H:3 * H]
    ee2 = np.asarray(ee2_w, f32)
    rev = np.asarray(rev_w, f32)
    eu2 = np.asarray(eu2_w, f32)
    ne = np.asarray(ne_w, f32)
    neb = np.asarray(ne_b, f32)
    rve = np.asarray(rve_w, f32)
    nu1 = np.asarray(nu1_w, f32)
    nu1a_m, nu1b_m = nu1[0:H], nu1[H:2 * H]
    nu3 = np.asarray(nu3_w, f32)
    dxw = np.asarray(dx_w, f32)
    sp = float(np.log1p(np.exp(np.float64(np.asarray(bf_raw)))))

    nerve = ne @ rve                      # (4, H)
    rtb = rve.T @ neb                     # (H,)
    wpv = np.zeros((H, 8), f32)
    wpv[:, 0] = neb
    wpv[:, 1] = U3.T @ rtb
    wpv[:, 2] = U2.T @ rtb
    wpv[:, 3] = (np.asarray(nu1_b, f32)
                 + (N - 1.0) * (nu1b_m.T @ (rev.T @ np.asarray(eu2_b, f32))))
    wpv[:, 4] = np.asarray(nu2_b, f32)
    wpv[0:D, 5] = np.asarray(dx_b, f32) + dxw.T @ np.asarray(nu3_b, f32)
    wpv[:, 6] = np.asarray(ee1_b, f32)
    wpv[:, 7] = U1.T @ np.asarray(ee2_b, f32) + np.asarray(eu1_b, f32)

    ee1pad = np.zeros((H, H), f32)
    ee1pad[0:5] = np.asarray(ee1_w, f32)
    wp32b = np.zeros((H, 390), f32)
    wp32b[:, 0:128] = eu2 @ rev @ nu1b_m
    wp32b[:, 128:256] = nu1a_m
    wp32b[:, 256:384] = np.asarray(nu2_w, f32)
    wp32b[:, 384:387] = nu3 @ dxw
    wp32b[:, 387:390] = dxw

    shared = {
        "ne3": np.concatenate([ne, nerve @ U3, nerve @ U2], axis=1),
        "wpv": wpv,
        "wpbf": np.concatenate([ee2 @ U1, np.eye(H, dtype=f32), ee1pad],
                               axis=1).astype(bf),
        "wp32b": wp32b,
        "_sp": sp,
    }

    in_maps = []
    for c in range(NCORES):
        xs = x[c * BPC:(c + 1) * BPC]          # (BPC, N, D)
        E = np.empty((BPC, 5, N * N), f32)
        for s in range(BPC):
            diff = xs[s][:, None, :] - xs[s][None, :, :]   # (i, j, d)
            r2 = np.sum(diff * diff, axis=-1)
            r1 = np.sqrt(r2 + EPS)
            E[s, 0:3] = diff.transpose(2, 0, 1).reshape(3, N * N)
            E[s, 3] = r1.reshape(N * N)
            E[s, 4] = r2.reshape(N * N)
        xT = np.ascontiguousarray(xs.transpose(0, 2, 1))   # (BPC, D, N)
        xt4 = np.concatenate(
            [xT, np.broadcast_to(spin_f[None, None, :], (BPC, 1, N))], axis=1)
        m = dict(shared)
        m["E"] = E.astype(bf)
        m["xt4"] = np.ascontiguousarray(
            np.concatenate([xt4[0], xt4[1]], axis=1))
        in_maps.append(m)
    return in_maps


def kernel(**inputs):
    from concourse.bass_utils import run_bass_kernel_spmd

    in_maps = _prep_inputs(**inputs)
    sp = in_maps[0].pop("_sp")
    for m in in_maps[1:]:
        m.pop("_sp")
    ck = ("nc", round(sp, 9))
    if ck not in _CACHE:
        _CACHE[ck] = _build_nc(sp)
    nc = _CACHE[ck]

    res = run_bass_kernel_spmd(nc, in_maps, core_ids=list(range(NCORES)))
    outs = [np.asarray(r["out"]).reshape(BPC, D, N) for r in res.results]
    full = np.concatenate(outs, axis=0)
    return np.ascontiguousarray(full.transpose(0, 2, 1)).astype(np.float32)
